# revision 16
# baseline (speedup 1.0000x reference)
"""Causal multi-head attention on 8 TRN2 NeuronCores — pipelined v2.

Problem: B=2, L=2048, H=16, E=64 (f32 in/out). B*H = 32 (batch, head)
slices are data-parallel: 4 slices per core, no cross-core comm.

Per-core, per-slice algorithm (matmul operands bf16, PSUM f32):
  - S^T row-blocks: for each m-tile mi, K^T(mi) stationary (duplicated
    into both partition halves -> contraction 128 computes 2*S, absorbed
    by halving the exp scale; keeps the PE activity monitor warm), Q^T
    streams the causal l-range in <=512-col matmuls into 2-bank PSUM
    chunks.
  - exp is split across two engines so neither is the critical path:
    ScalarE runs native Exp; VectorE runs a Schraudolph-style exp
    (int16(A*s + B) bitcast as bf16 == 2^(A*s+B-16256 mantissa interp)),
    ~ +/-2% sawtooth on ~45% of weights, which the softmax weighted
    average attenuates to ~1e-2 output rel err.
  - causal mask on the diagonal block of each row via gpsimd
    affine_select (fill 0 on the exp'd weights).
  - PV: per-li chains, P^T block stationary, V streamed (65 cols: 64 V
    columns + a ones column that accumulates the softmax denominator),
    accumulated over mi into a [128, 4*65] PSUM bank per 4-li group.
  - normalize: one reciprocal [128,4] + one broadcast tensor_tensor
    multiply per group -> o_sb f32 -> contiguous DMA out.
O-chains are emitted with a 2-row lag behind S rows so the PE never
waits on exp (stays dense -> HAM stays at full clock).
"""

import numpy as np
import ml_dtypes
from contextlib import ExitStack

import concourse.bass as bass
import concourse.mybir as mybir
import concourse.tile as tile
from concourse import bacc
from concourse.bass_utils import run_bass_kernel_spmd

B, L, H, E = 2, 2048, 16, 64
N_CORES = 8
NS = (B * H) // N_CORES  # slices per core = 4
NT = L // 128  # 16 tiles of 128 along both l and m
SCALE = 0.0625  # 1/sqrt(E) / 2 (K-duplicated S matmul computes 2*S)
# Schraudolph exp for the VectorE share: bf16 bits of exp(SCALE*s) are
# approximately int16(EXP_A*s + EXP_B) (linear mantissa interpolation).
EXP_A = 128 * SCALE * 1.4426950408889634  # 8*log2(e)
EXP_B = 16249.5  # 127*128 + offset tuned for ~zero-mean sawtooth
F32 = mybir.dt.float32
BF16 = mybir.dt.bfloat16
I16 = mybir.dt.int16
BF16NP = ml_dtypes.bfloat16

# pT column base of row-block mi (rows packed back-to-back, 128 cols/block)
ROW_BASE = [128 * (16 * k - (k * (k - 1)) // 2) for k in range(NT + 1)]
PT_COLS = ROW_BASE[NT]  # 17408


def _plan():
    """Static per-slice chunk schedule.

    Row k (m-tile k) covers l-tiles [k, 16). It is split into chunk 0
    (first min(8, 16-k) blocks) and, for k<8, chunk 1 (the rest). Each
    chunk lands in a fresh 2-bank PSUM tile and is exp'd by a statically
    assigned engine (greedy balance of estimated busy time).
    """
    chunks = []  # (k, ci, nblk, engine)
    act_t, dve_t = 0.0, 0.0
    for k in range(NT):
        nblk = NT - k
        clist = [(k, 0, min(8, nblk))]
        if nblk > 8:
            clist.append((k, 1, nblk - 8))
        for kk, ci, cb in clist:
            cols = 128 * cb
            ea = (cols + 352) / 1.2
            ed = cols / 0.96 + 150.0
            if act_t + ea <= dve_t + ed:
                eng = "act"
                act_t += ea
            else:
                eng = "dve"
                dve_t += ed
            chunks.append((kk, ci, cb, eng))
        if k % 4 == 3:
            dve_t += 600.0  # reciprocal + broadcast multiply share
    return chunks


CHUNKS = _plan()
CHUNKS_BY_ROW = {}
for _c in CHUNKS:
    CHUNKS_BY_ROW.setdefault(_c[0], []).append(_c)


def _emit_slice(tc, pools, qT, kT, v, outT, s):
    """Generator: yields after (1) DMA issue, (2) rows 0-1, (3) body part 1
    (rows 2-8), (4) body part 2 (rows 9-15, chains to O(13)), (5) O(14).
    The driver weaves these across slices so the next slice's loads are
    issued half a slice early and its first S rows interleave with this
    slice's tail O-chains (keeps exp engines fed at slice boundaries)."""
    nc = tc.nc
    io_q, io_k, io_v, pt_pool, r_pool, nm_pool, psS, psO, wu_pool = pools

    # Q^T/K^T duplicated into both partition halves (contraction 128).
    qT_sb = io_q.tile([128, L], BF16, name="qs", tag="qs")
    kT_sb = io_k.tile([128, L], BF16, name="ks", tag="ks")
    v_sb = io_v.tile([128, NT * 65], BF16, name="vs", tag="vs")
    if s == 0:
        # First slice: dummy matmuls on a zeroed tile bridge the DMA head
        # so the PE activity monitor reaches full clock by the time real
        # operands land; loads are chunked in consumption order.
        wu_sb = wu_pool.tile([128, 512], BF16, name="wu", tag="wu")
        nc.gpsimd.memset(wu_sb[:, :], 0.0)
        wu_ps = psO.tile([128, 512], F32, name="po", tag="po")
        for _ in range(10):
            nc.tensor.matmul(
                wu_ps[:, 0:512],
                lhsT=wu_sb[:, 0:128],
                rhs=wu_sb[:, 0:512],
                start=True,
                stop=True,
                skip_group_check=True,
            )
        # dual-ring loads (sync + scalar DGE) in consumption order
        c0, c1 = slice(0, 1024), slice(1024, 2048)
        nc.sync.dma_start(kT_sb[0:E, c0], kT[s][:, c0])
        nc.scalar.dma_start(kT_sb[E:128, c0], kT[s][:, c0])
        nc.sync.dma_start(qT_sb[0:E, c0], qT[s][:, c0])
        nc.scalar.dma_start(qT_sb[E:128, c0], qT[s][:, c0])
        nc.sync.dma_start(qT_sb[0:E, c1], qT[s][:, c1])
        nc.scalar.dma_start(qT_sb[E:128, c1], qT[s][:, c1])
        nc.scalar.dma_start(v_sb[:, :], v[s])
        nc.sync.dma_start(kT_sb[0:E, c1], kT[s][:, c1])
        nc.scalar.dma_start(kT_sb[E:128, c1], kT[s][:, c1])
    else:
        # inputs split across the two DGE rings so input + output traffic
        # fits the slice period (one ring alone saturates)
        nc.sync.dma_start(qT_sb[0:E, :], qT[s])
        nc.scalar.dma_start(qT_sb[E:128, :], qT[s])
        nc.sync.dma_start(kT_sb[0:E, :], kT[s])
        nc.scalar.dma_start(kT_sb[E:128, :], kT[s])
        nc.scalar.dma_start(v_sb[:, :], v[s])

    pT = pt_pool.tile([128, PT_COLS], BF16, name="pt", tag="pt")

    po = {}

    def pv_block(li, mi):
        lp, c = li // 4, li % 4
        if lp not in po:
            po[lp] = psO.tile([128, 4 * 65], F32, name="po", tag="po")
        blk = ROW_BASE[mi] + 128 * (li - mi)
        nc.tensor.matmul(
            po[lp][:, 65 * c : 65 * c + 65],
            lhsT=pT[:, blk : blk + 128],
            rhs=v_sb[:, 65 * mi : 65 * mi + 65],
            start=(mi == 0),
            stop=(mi == li),
            skip_group_check=True,
        )

    def finish_group(li):
        # normalize the completed 4-li group: reciprocal of the ones-column
        # denominators (DVE), broadcast multiply (gpsimd), DMA out
        lp = li // 4
        pot = po[lp]
        r_sb = r_pool.tile([128, 4], F32, name="rr", tag="rr")
        den = pot.rearrange("p (c x) -> p c x", c=4, x=65)[:, :, 64]
        nc.vector.reciprocal(r_sb[:, :], den)
        ob = nm_pool.tile([128, 4 * E], F32, name="ot", tag="ot")
        nc.vector.tensor_tensor(
            ob.rearrange("p (c e) -> p c e", c=4, e=E),
            pot.rearrange("p (c x) -> p c x", c=4, x=65)[:, :, 0:E],
            r_sb[:, :, None].to_broadcast([128, 4, E]),
            mybir.AluOpType.mult,
        )
        nc.sync.dma_start(outT[s, lp], ob)
        del po[lp]

    def s_chunk(k, ci, cb, eng):
        ps = psS.tile([128, 1024], F32, name="ps", tag="ps")
        l0 = k + 8 * ci
        off = 0
        while off < cb:
            n = min(4, cb - off)
            nc.tensor.matmul(
                ps[:, 128 * off : 128 * (off + n)],
                lhsT=kT_sb[:, 128 * k : 128 * k + 128],
                rhs=qT_sb[:, 128 * (l0 + off) : 128 * (l0 + off + n)],
                start=True,
                stop=True,
            )
            off += n
        cols = 128 * cb
        pt_off = ROW_BASE[k] + 1024 * ci
        if eng == "act":
            nc.scalar.activation(
                pT[:, pt_off : pt_off + cols],
                ps[:, :cols],
                mybir.ActivationFunctionType.Exp,
                scale=SCALE,
            )
        else:
            nc.vector.tensor_scalar(
                pT[:, pt_off : pt_off + cols].bitcast(I16),
                ps[:, :cols],
                EXP_A,
                EXP_B,
                mybir.AluOpType.mult,
                mybir.AluOpType.add,
            )
        if ci == 0:
            # causal mask on the diagonal block: keep m <= l'
            seg = pT[:, ROW_BASE[k] : ROW_BASE[k] + 128]
            nc.gpsimd.affine_select(
                out=seg,
                in_=seg,
                pattern=[[1, 128]],
                compare_op=mybir.AluOpType.is_ge,
                fill=0.0,
                base=0,
                channel_multiplier=-1,
            )

    yield  # loads issued
    for k in (0, 1):
        for ck in CHUNKS_BY_ROW[k]:
            s_chunk(*ck)
    yield  # head rows done
    for k in range(2, NT):
        for ck in CHUNKS_BY_ROW[k]:
            s_chunk(*ck)
        li = k - 2
        for mi in range(li + 1):  # mi ascending; diag (freshest exp) last
            pv_block(li, mi)
        if li % 4 == 3:
            finish_group(li)
        if k == 6:
            yield  # mid-body: driver issues next slice's loads here
    yield  # body done (chains through O(13))
    for mi in range(NT - 1):
        pv_block(NT - 2, mi)
    yield  # O(14) done
    for mi in range(NT):
        pv_block(NT - 1, mi)
    finish_group(NT - 1)


def _build():
    nc = bacc.Bacc(
        "TRN2",
        target_bir_lowering=False,
        debug=False,
        enable_asserts=True,
        num_devices=N_CORES,
    )
    qT = nc.dram_tensor("qT", [NS, E, L], BF16, kind="ExternalInput").ap()
    kT = nc.dram_tensor("kT", [NS, E, L], BF16, kind="ExternalInput").ap()
    v = nc.dram_tensor("v", [NS, 128, NT * 65], BF16, kind="ExternalInput").ap()
    outT = nc.dram_tensor("outT", [NS, 4, 128, 4 * E], F32, kind="ExternalOutput").ap()

    with tile.TileContext(nc) as tc:
        with ExitStack() as ctx:

            def pool(name, bufs, space="SBUF"):
                return ctx.enter_context(
                    tc.tile_pool(name=name, bufs=bufs, space=space)
                )

            pools = (
                pool("io_q", 2),
                pool("io_k", 2),
                pool("io_v", 2),
                pool("pt", 2),
                pool("r", 4),
                pool("nm", 2),
                pool("psS", 3, "PSUM"),
                pool("psO", 2, "PSUM"),
                pool("wu", 1),
            )

            def adv(g):
                try:
                    next(g)
                except StopIteration:
                    pass

            gens = [
                _emit_slice(tc, pools, qT, kT, v, outT, s) for s in range(NS)
            ]
            adv(gens[0])  # dmas s0
            adv(gens[0])  # rows 0-1 s0
            for s in range(NS):
                adv(gens[s])  # body part 1 (rows 2-8)
                if s + 1 < NS:
                    adv(gens[s + 1])  # issue s+1 loads half a slice early
                adv(gens[s])  # body part 2 (rows 9-15)
                adv(gens[s])  # O(14)
                if s + 1 < NS:
                    adv(gens[s + 1])  # s+1 rows 0-1 between the tails
                adv(gens[s])  # O(15) + finish

    nc.compile()
    return nc


_NC_CACHE = {}


def _get_nc():
    if "nc" not in _NC_CACHE:
        _NC_CACHE["nc"] = _build()
    return _NC_CACHE["nc"]


def kernel(queries, keys, values, trace=False, tmpdir=None):
    nc = _get_nc()

    BH = B * H
    # shard: slice g = b*H + h; per-core slices [4c, 4c+4)
    qTf = np.ascontiguousarray(
        queries.transpose(0, 2, 3, 1).reshape(BH, E, L)
    ).astype(BF16NP)
    kTf = np.ascontiguousarray(
        keys.transpose(0, 2, 3, 1).reshape(BH, E, L)
    ).astype(BF16NP)
    # V in SBUF layout [128, 16*65]: cols 65t..65t+63 = V rows 128t+p,
    # col 65t+64 = 1.0 (softmax denominator trick)
    v4 = values.transpose(0, 2, 1, 3).reshape(BH, NT, 128, E)  # [g, t, p, e]
    varr = np.ones((BH, 128, NT, 65), dtype=BF16NP)
    varr[:, :, :, :E] = np.asarray(v4, dtype=np.float32).transpose(0, 2, 1, 3)
    vf = np.ascontiguousarray(varr.reshape(BH, 128, NT * 65))

    in_maps = [
        {
            "qT": qTf[NS * c : NS * (c + 1)],
            "kT": kTf[NS * c : NS * (c + 1)],
            "v": vf[NS * c : NS * (c + 1)],
        }
        for c in range(N_CORES)
    ]

    res = run_bass_kernel_spmd(
        nc, in_maps, core_ids=list(range(N_CORES)), trace=trace, tmpdir=tmpdir
    )

    outT = np.concatenate([res.results[c]["outT"] for c in range(N_CORES)], axis=0)
    # outT: [BH, lp, p, c*64+e] with l = 512*lp + 128*c + p
    out = (
        outT.reshape(BH, 4, 128, 4, E)
        .transpose(0, 1, 3, 2, 4)
        .reshape(B, H, L, E)
        .transpose(0, 2, 1, 3)
    )
    out = np.ascontiguousarray(out, dtype=np.float32)
    if trace:
        kernel.last_exec_time_ns = res.exec_time_ns
    return out
